# revision 51
# baseline (speedup 1.0000x reference)
"""Trainium2 Bass kernel for nn_MultiHeadAttention_21251498181338.

Music-Transformer-style MHA with relative position embeddings (Huang et al.
skew trick), B=2, L=2048, D=1024, H=16, causal mask.

Sharding: 8 cores = 2 batches x 4-head groups (tensor parallel per head).
Each core computes q/k/v projections for its 4 heads, causal attention with
relative position logits, and a partial output projection (Wo row-split).
Partials are summed on the host during unshard.

Device-side structure (per core):
  - Projections produce qh^T/kh^T in [head-depth on partitions] layout and
    vh in [keys on partitions] layout, so no transposes are needed anywhere
    except for the attention probabilities themselves.
  - P = exp(QK^T/8) * exp(Srel/8): the additive logit split is computed
    multiplicatively so the relative-position term can be skew-aligned
    independently of QK^T.
  - The skew is a single SBUF->SBUF DMA per (head, q-tile) using a flat
    access pattern with partition step (row_len - 1): row i is read with a
    column offset of -i, which is exactly the Huang et al. pad/reshape
    trick. Columns beyond the valid relative-index range are zeroed, which
    also implements the causal mask for free (P = Pqk * 0 = 0 there).
  - PV uses TensorE transposes of P tiles. The softmax denominators come
    for free from the fused multiply+reduce (tensor_tensor_reduce): the
    per-query row sums accumulate on the vector engine during the
    P = exp(QK)*exp(Srel) multiply, so no denominator matmuls are needed.
    The per-query reciprocals are turned into a partition-replicated
    [128, 128] tile via a DVE 32x32 stream-transpose plus two tiny
    SBUF->SBUF broadcast DMAs.
  - The two heads of each pair interleave their K=64 matmuls (different PE
    row-groups run concurrently) and share [128, P] psum tiles for PV via
    tile_position column halves, so both heads normalize in one op and
    land directly in the packed outT layout.
  - The attention output appears transposed [depth, queries], which is
    exactly the stationary-operand layout the output projection needs.
"""

import os
import sys

sys.path.insert(0, "/opt/trn_rl_repo")

import numpy as np
import ml_dtypes

import concourse.bass as bass
import concourse.mybir as mybir
import concourse.tile as tile
from concourse import bacc
from concourse.bass_utils import run_bass_kernel_spmd
from concourse.masks import make_identity

BF16 = mybir.dt.bfloat16
F32 = mybir.dt.float32
NPBF16 = ml_dtypes.bfloat16

B, L, DM, H, D = 2, 2048, 1024, 16, 64
HG = 4            # heads per core (head group)
NCORES = 8
P = 128
KT = DM // P      # 8 contraction tiles for projections
NIT = L // P      # 16 query tiles
SCALE = 1.0 / np.sqrt(D)  # 0.125

LAST_EXEC_NS = None

_PROG = None


def _ncj(it):
    # number of 512-wide key chunks for query tile `it` (causal)
    return it // 4 + 1


def build_program():
    nc = bacc.Bacc(
        "TRN2",
        target_bir_lowering=False,
        debug=False,
        enable_asserts=False,
        num_devices=NCORES,
    )

    # ---- External I/O ----
    xq = nc.dram_tensor("xq", [DM, L], BF16, kind="ExternalInput")  # q[b].T
    xk = nc.dram_tensor("xk", [DM, L], BF16, kind="ExternalInput")
    xv = nc.dram_tensor("xv", [DM, L], BF16, kind="ExternalInput")
    wq = nc.dram_tensor("wq", [DM, 2 * P], BF16, kind="ExternalInput")  # group cols
    wk = nc.dram_tensor("wk", [DM, 2 * P], BF16, kind="ExternalInput")
    wv = nc.dram_tensor("wv", [DM, 2 * P], BF16, kind="ExternalInput")
    wo = nc.dram_tensor("wo", [2, P, DM], BF16, kind="ExternalInput")  # [hp, 2h*64, dm]
    eT = nc.dram_tensor("eT", [2, P, L], BF16, kind="ExternalInput")   # [hp, 2h*64, r]
    bqk = nc.dram_tensor("bqk", [P, 6], F32, kind="ExternalInput")  # bq|bk|bv
    bo_t = nc.dram_tensor("bo", [P, DM], F32, kind="ExternalInput")     # row-replicated
    out = nc.dram_tensor("out", [L, DM], F32, kind="ExternalOutput")

    with tile.TileContext(nc) as tc:
        with (
            tc.tile_pool(name="persist", bufs=1) as pp,
            tc.tile_pool(name="work", bufs=1) as wp,
            tc.tile_pool(name="small", bufs=4) as sp,
        ):
            # ---- persistent SBUF tensors ----
            ident = pp.tile([P, P], BF16)
            make_identity(nc, ident)

            wq_sb = pp.tile([P, KT, 2 * P], BF16)
            nc.sync.dma_start(wq_sb, wq.ap().rearrange("(t p) c -> p t c", p=P))
            wk_sb = pp.tile([P, KT, 2 * P], BF16)
            nc.sync.dma_start(wk_sb, wk.ap().rearrange("(t p) c -> p t c", p=P))
            wv_sb = pp.tile([P, KT, 2 * P], BF16)
            nc.sync.dma_start(wv_sb, wv.ap().rearrange("(t p) c -> p t c", p=P))
            wo_sb = pp.tile([P, 2, DM], BF16)
            nc.sync.dma_start(wo_sb, wo.ap().rearrange("h p m -> p h m"))
            eT_sb = pp.tile([P, 2, L], BF16)
            nc.sync.dma_start(eT_sb, eT.ap().rearrange("h p r -> p h r"))
            bqk_sb = pp.tile([P, 6], F32)
            nc.sync.dma_start(bqk_sb, bqk.ap())
            bo_sb = pp.tile([P, DM], F32)
            nc.sync.dma_start(bo_sb, bo_t.ap())

            qhT = pp.tile([P, 2, L], BF16)   # [64*hl+d, hp, i]
            khT = pp.tile([P, 2, L], BF16)
            vhT = pp.tile([P, 2, L], BF16)
            vh = pp.tile([P, NIT, HG, 66], BF16)  # [j in tile, jt, local head, d|1|pad]
            outT = pp.tile([P, 2, L], BF16)  # [64*hl+d, hp, i]

            # exp(Srel) band buffers, one per (head, tile-in-group): each
            # use writes exp values into [0:W] and memsets the zero tail
            # [W:nj+128]; the diagonal skew DMA never reads past nj+127.
            XSE_W = 2176  # 2048 max band + 128 skew overhang
            xse_pp = [[pp.tile([P, XSE_W], BF16, name=f"xse{hl}_{i}")
                       for i in range(4)] for hl in range(2)]
            # all-ones stationary for the softmax-denominator matmul:
            # lhsT [128, 64] of ones -> psum rows all equal to the denom
            ones64 = pp.tile([P, 64], BF16)
            nc.gpsimd.memset(ones64, 1.0)

            # ---- Stage 1: projections ----
            # All three projections use the same dense N=512 streams with
            # the weight slice stationary, producing [head-depth on
            # partitions] outputs; vh's [keys on partitions] layout is then
            # recovered with 32 PE transposes.
            with (
                tc.tile_pool(name="xin", bufs=9) as xp,
                tc.tile_pool(name="ps1", bufs=4, space="PSUM") as ps1,
                tc.tile_pool(name="psvT", bufs=2, space="PSUM") as psvTp,
            ):
                stage1 = ((xv, wv_sb, vhT, 4), (xk, wk_sb, khT, 2),
                          (xq, wq_sb, qhT, 0))
                for src, wsb, dst, bcol in stage1:
                    # per-kt chunk DMAs: matmuls start as soon as the first
                    # chunk lands instead of waiting for the full tensor
                    chunks = []
                    for kt in range(KT):
                        ch = xp.tile([P, L], BF16, tag="xchunk")
                        nc.sync.dma_start(ch, src.ap()[kt * P:(kt + 1) * P, :])
                        chunks.append(ch)
                    # kt-outer per hp: one LDWEIGHTS feeds 4 back-to-back
                    # matmuls; 4 psum tiles accumulate in parallel
                    for hp in range(2):
                        pss = {}
                        for ic in range(L // 512):
                            pss[ic] = ps1.tile([P, 512], F32, tag="ps1",
                                               name=f"ps1_{ic}")
                        for kt in range(KT):
                            for ic in range(L // 512):
                                nc.tensor.matmul(
                                    pss[ic],
                                    wsb[:, kt, hp * P:(hp + 1) * P],
                                    chunks[kt][:, ic * 512:(ic + 1) * 512],
                                    start=(kt == 0),
                                    stop=(kt == KT - 1),
                                )
                        for ic in range(L // 512):
                            nc.vector.tensor_scalar_add(
                                dst[:, hp, ic * 512:(ic + 1) * 512],
                                pss[ic],
                                bqk_sb[:, bcol + hp:bcol + hp + 1],
                            )
                    if src is xk:
                        # vhT -> vh transposes, emitted here so the k-stream
                        # masks the wait on vhT's bias adds
                        for hp in range(2):
                            for jt in range(NIT):
                                pvt = psvTp.tile([P, P], BF16, tag="psvT")
                                nc.tensor.transpose(
                                    pvt, vhT[:, hp, jt * P:(jt + 1) * P],
                                    ident)
                                nc.vector.tensor_copy(
                                    vh[:, jt, 2 * hp:2 * hp + 2, 0:64],
                                    pvt.rearrange("p (l d) -> p l d", l=2),
                                )

            # ---- Stage 2: attention, 4-q-tile supertiles ----
            # The four q-tiles of group g share the same chunk count (g+1),
            # so P^T is assembled per key-tile as a [128, 512] moving
            # operand: PV and the ones-denominator run as N=512 streams (4x
            # fewer matmuls). Masked/overhang blocks are exact zeros via the
            # skewed-band zero tail. The two heads of a pair interleave
            # their K=64 matmuls (different PE row-groups run concurrently).
            with (
                tc.tile_pool(name="psA", bufs=3, space="PSUM") as psAp,
                tc.tile_pool(name="psT", bufs=2, space="PSUM") as psTp,
                tc.tile_pool(name="psO", bufs=2, space="PSUM") as psOp,
                tc.tile_pool(name="psD", bufs=1, space="PSUM") as psDp,
            ):
                NG = NIT // 4
                # big and small groups interleaved: every scheduling window
                # contains one dense matmul stream, so the PE stays busy
                # (and HAM stays unthrottled) while the small groups' long
                # scalar/DMA chains drain.
                g_order = [NG - 1, 0, NG - 2, 1]
                for g in g_order:
                    nj = 512 * (g + 1)         # group rectangle width
                    nkt = 4 * (g + 1)          # key tiles in the rectangle
                    for hp in range(2):
                        pm = [[None] * 4 for _ in range(2)]
                        for i in range(4):
                            it = 4 * g + i
                            i0 = it * P
                            W = (it + 1) * P   # valid band width
                            r_lo = L - P - i0  # first rel index in band
                            q_stat = [qhT[64 * hl:64 * hl + 64, hp,
                                          i0:i0 + P] for hl in (0, 1)]
                            xse = [xse_pp[hl][i] for hl in (0, 1)]
                            for hl in (0, 1):
                                # zero tail [W:nj+128]: masks the causal
                                # region and pads the rectangle
                                nc.gpsimd.memset(xse[hl][:, W:nj + P], 0.0)
                            # exp(Srel/8) band [query on part, r on free]
                            for cs in range(g + 1):
                                n = min(512, W - cs * 512)
                                for hl in (0, 1):
                                    pb = 64 * hl
                                    ps = psAp.tile([P, 512], F32, tag="psA")
                                    nc.tensor.matmul(
                                        ps[:, :n],
                                        q_stat[hl],
                                        eT_sb[pb:pb + 64, hp,
                                              r_lo + cs * 512:
                                              r_lo + cs * 512 + n],
                                        start=True, stop=True,
                                    )
                                    nc.scalar.activation(
                                        xse[hl][:, cs * 512:cs * 512 + n],
                                        ps[:, :n],
                                        mybir.ActivationFunctionType.Exp,
                                        scale=SCALE,
                                    )
                            # exp(QK^T/8) chunks, skew chunks, and the
                            # P = pqk * xsk multiply, all 512 wide
                            for hl in (0, 1):
                                pm[hl][i] = wp.tile([P, 2048], BF16,
                                                    tag=f"pm{hl}_{i}",
                                                    name=f"pm{hl}_{i}")
                            for jc in range(g + 1):
                                for hl in (0, 1):
                                    pb = 64 * hl
                                    ps = psAp.tile([P, 512], F32, tag="psA")
                                    nc.tensor.matmul(
                                        ps,
                                        q_stat[hl],
                                        khT[pb:pb + 64, hp,
                                            jc * 512:(jc + 1) * 512],
                                        start=True, stop=True,
                                    )
                                    pqc = sp.tile([P, 512], BF16,
                                                  tag=f"pqc{hl}")
                                    nc.scalar.activation(
                                        pqc, ps,
                                        mybir.ActivationFunctionType.Exp,
                                        scale=SCALE,
                                    )
                                    row_len = xse[hl].ap[0][0]
                                    diag = bass.AP(
                                        xse[hl].tensor,
                                        xse[hl].offset + 127 + jc * 512,
                                        [[row_len - 1, P], [1, 512]],
                                    )
                                    xkc = sp.tile([P, 512], BF16,
                                                  tag=f"xkc{hl}")
                                    nc.sync.dma_start(xkc, diag)
                                    nc.vector.tensor_tensor(
                                        pm[hl][i][:, jc * 512:(jc + 1) * 512],
                                        pqc, xkc, mybir.AluOpType.mult,
                                    )

                        # P^T per key tile as [128 k, 512 q] -> N=512 PV and
                        # denominator streams. Head hl owns psum partitions
                        # [64*hl, 64*hl+64); pso/psd each own a full bank
                        # (a bank tolerates only ONE open accumulation group
                        # per partition range).
                        pso = psOp.tile([P, 512], F32, tag="psO")
                        psd = psDp.tile([P, 512], F32, tag="psD")
                        for jt in range(nkt):
                            for hl in (0, 1):
                                pb = 64 * hl
                                lh = 2 * hp + hl
                                pst = psTp.tile([P, 512], BF16, tag="psT")
                                for i in range(4):
                                    nc.tensor.transpose(
                                        pst[:, i * P:(i + 1) * P],
                                        pm[hl][i][:, jt * P:(jt + 1) * P],
                                        ident,
                                    )
                                pts = sp.tile([P, 512], BF16, tag="pts")
                                nc.vector.tensor_copy(pts, pst)
                                nc.tensor.matmul(
                                    pso[pb:pb + 64, :],
                                    vh[:, jt, lh, 0:64],
                                    pts,
                                    start=(jt == 0),
                                    stop=(jt == nkt - 1),
                                    skip_group_check=True,
                                )
                                nc.tensor.matmul(
                                    psd[pb:pb + 64, :],
                                    ones64,
                                    pts,
                                    start=(jt == 0),
                                    stop=(jt == nkt - 1),
                                    skip_group_check=True,
                                )

                        # normalize both heads at once; writes land directly
                        # in the packed outT partition halves
                        rrec = sp.tile([P, 512], F32, tag="rrec")
                        nc.vector.reciprocal_approx_fast(out=rrec, in_=psd)
                        nc.vector.tensor_tensor(
                            outT[:, hp, 4 * g * P:4 * g * P + 512],
                            pso, rrec, mybir.AluOpType.mult,
                        )

            # ---- Stage 3: output projection (partial: this head group) ----
            with tc.tile_pool(name="ps3", bufs=2, space="PSUM") as ps3:
                for it in range(NIT):
                    pss3 = [ps3.tile([P, 512], F32, tag=f"ps3{mc}",
                                     name=f"ps3{mc}")
                            for mc in range(DM // 512)]
                    for hp in range(2):
                        for mc in range(DM // 512):
                            nc.tensor.matmul(
                                pss3[mc],
                                outT[:, hp, it * P:(it + 1) * P],
                                wo_sb[:, hp, mc * 512:(mc + 1) * 512],
                                start=(hp == 0),
                                stop=(hp == 1),
                            )
                    for mc in range(DM // 512):
                        osb = sp.tile([P, 512], F32, tag="osb")
                        nc.vector.tensor_tensor(
                            osb, pss3[mc], bo_sb[:, mc * 512:(mc + 1) * 512],
                            mybir.AluOpType.add,
                        )
                        nc.sync.dma_start(
                            out.ap()[it * P:(it + 1) * P, mc * 512:(mc + 1) * 512], osb
                        )
    nc.compile()
    return nc


def _prep_inputs(q, k, v, Wq, bq, Wk, bk, Wv, bv, Wo, bo, E):
    """Build the 8 per-core input maps (host-side shard + cast)."""
    in_maps = []
    for core in range(NCORES):
        b, g = core // HG, core % HG
        cols = slice(g * HG * D, (g + 1) * HG * D)  # this group's 256 cols
        # eT/wo packing: [hp, 64*hl + d, .]
        eTg = np.empty((2, P, L), NPBF16)
        wog = np.empty((2, P, DM), NPBF16)
        for hp in range(2):
            for hl in range(2):
                h = g * HG + 2 * hp + hl
                eTg[hp, 64 * hl:64 * hl + 64, :] = E[:, h * D:(h + 1) * D].T.astype(NPBF16)
                wog[hp, 64 * hl:64 * hl + 64, :] = Wo[h * D:(h + 1) * D, :].astype(NPBF16)
        bqk_a = np.empty((P, 6), np.float32)
        for hp in range(2):
            sl = slice(g * HG * D + hp * P, g * HG * D + (hp + 1) * P)
            bqk_a[:, hp] = bq[sl]
            bqk_a[:, 2 + hp] = bk[sl]
            bqk_a[:, 4 + hp] = bv[sl]
        bo_full = bo if g == 0 else np.zeros_like(bo)
        in_maps.append({
            "xq": np.ascontiguousarray(q[b].T).astype(NPBF16),
            "xk": np.ascontiguousarray(k[b].T).astype(NPBF16),
            "xv": np.ascontiguousarray(v[b].T).astype(NPBF16),
            "wq": np.ascontiguousarray(Wq[:, cols]).astype(NPBF16),
            "wk": np.ascontiguousarray(Wk[:, cols]).astype(NPBF16),
            "wv": np.ascontiguousarray(Wv[:, cols]).astype(NPBF16),
            "wo": wog,
            "eT": eTg,
            "bqk": bqk_a,
            "bo": np.ascontiguousarray(
                np.broadcast_to(bo_full[None, :], (P, DM))).astype(np.float32),
        })
    return in_maps


def _reference_numpy(q, k, v, mask, Wq, bq, Wk, bk, Wv, bv, Wo, bo, E):
    """Exact fallback for non-causal masks (never hit in practice)."""
    def split_heads(x):
        return np.moveaxis(x.reshape(*x.shape[:-1], H, D), -2, -3)
    qh = split_heads(q @ Wq + bq)
    kh = split_heads(k @ Wk + bk)
    vv = split_heads(v @ Wv + bv)
    eh = split_heads(E)
    QKt = np.einsum("bhqd,bhkd->bhqk", qh, kh)
    X = np.einsum("bhqd,hkd->bhqk", qh, eh)
    pad = np.pad(X, [(0, 0)] * 3 + [(1, 0)])
    s = pad.reshape(B, H, -1)[:, :, L:].reshape(B, H, L, L)
    logits = (QKt + s) / np.sqrt(D) + mask * -1e9
    m = logits.max(-1, keepdims=True)
    p = np.exp(logits - m)
    p /= p.sum(-1, keepdims=True)
    o = np.einsum("bhqk,bhkd->bhqd", p, vv)
    o = np.moveaxis(o, -3, -2).reshape(B, L, DM)
    return (o @ Wo + bo).astype(np.float32)


def benchmark(inputs, iters=20):
    """Amortized wall-clock of the sharded NEFF execution (device-resident
    inputs, back-to-back async dispatch). Returns est. ns per execution."""
    global _PROG
    import time as _time
    import jax
    from jax.sharding import Mesh, PartitionSpec
    from jax.experimental.shard_map import shard_map
    import concourse.bass2jax as b2j
    import concourse.mybir as mb

    if _PROG is None:
        _PROG = build_program()
    nc = _PROG
    args = {n: np.asarray(inputs[n], np.float32)
            for n in ("q", "k", "v", "Wq", "bq", "Wk", "bk", "Wv", "bv",
                      "Wo", "bo", "E")}
    in_maps = _prep_inputs(**args)
    b2j.install_neuronx_cc_hook()

    partition_name = (nc.partition_id_tensor.name
                      if nc.partition_id_tensor else None)
    in_names, out_names, out_avals, zero_outs = [], [], [], []
    for alloc in nc.m.functions[0].allocations:
        if not isinstance(alloc, mb.MemoryLocationSet):
            continue
        name = alloc.memorylocations[0].name
        if alloc.kind == "ExternalInput":
            if name != partition_name:
                in_names.append(name)
        elif alloc.kind == "ExternalOutput":
            out_names.append(name)
            shape = tuple(alloc.tensor_shape)
            dtype = mb.dt.np(alloc.dtype)
            out_avals.append(jax.core.ShapedArray(shape, dtype))
            zero_outs.append(np.zeros(shape, dtype))
    n_params = len(in_names)
    n_outs = len(out_avals)
    all_names = in_names + out_names
    if partition_name is not None:
        all_names = all_names + [partition_name]

    def _body(*fargs):
        operands = list(fargs)
        if partition_name is not None:
            operands.append(b2j.partition_id_tensor())
        outs = b2j._bass_exec_p.bind(
            *operands, out_avals=tuple(out_avals), in_names=tuple(all_names),
            out_names=tuple(out_names), lowering_input_output_aliases=(),
            sim_require_finite=True, sim_require_nnan=True, nc=nc)
        return tuple(outs)

    devices = jax.devices()[:NCORES]
    mesh = Mesh(np.asarray(devices), ("core",))
    in_specs = (PartitionSpec("core"),) * (n_params + n_outs)
    out_specs = (PartitionSpec("core"),) * n_outs
    sharded = jax.jit(
        shard_map(_body, mesh=mesh, in_specs=in_specs, out_specs=out_specs,
                  check_rep=False),
        keep_unused=True)

    concat_in = [np.concatenate([np.asarray(in_maps[c][n])
                                 for c in range(NCORES)], axis=0)
                 for n in in_names]
    dev_in = [jax.device_put(a) for a in concat_in]
    concat_zero = [np.concatenate([z] * NCORES, axis=0) for z in zero_outs]

    dev_zero = [jax.device_put(z) for z in concat_zero]
    # warmup (compiles / caches)
    outs = sharded(*dev_in, *dev_zero)
    jax.block_until_ready(outs)

    t0 = _time.perf_counter()
    results = []
    for _ in range(iters):
        results.append(sharded(*dev_in, *dev_zero))
    jax.block_until_ready(results)
    t1 = _time.perf_counter()
    return (t1 - t0) / iters * 1e9


def kernel(**inputs):
    global _PROG, LAST_EXEC_NS
    args = {n: np.asarray(inputs[n], np.float32)
            for n in ("q", "k", "v", "Wq", "bq", "Wk", "bk", "Wv", "bv",
                      "Wo", "bo", "E")}
    mask = np.asarray(inputs["mask"], np.float32)

    causal = np.array_equal(mask, np.triu(np.ones((L, L), np.float32), k=1))
    if not causal:
        return _reference_numpy(mask=mask, **args)

    if _PROG is None:
        _PROG = build_program()
    in_maps = _prep_inputs(**args)
    trace = os.environ.get("KERNEL_TRACE", "0") == "1"
    try:
        res = run_bass_kernel_spmd(_PROG, in_maps, core_ids=list(range(NCORES)),
                                   trace=trace)
    except ModuleNotFoundError:
        # axon NTFF profiling hook unavailable in this container
        res = run_bass_kernel_spmd(_PROG, in_maps, core_ids=list(range(NCORES)),
                                   trace=False)
    LAST_EXEC_NS = res.exec_time_ns
    globals()["LAST_RESULTS"] = res

    full = np.zeros((B, L, DM), np.float32)
    for core in range(NCORES):
        full[core // HG] += res.results[core]["out"]
    return full



# revision 57
# speedup vs baseline: 1.0289x; 1.0289x over previous
"""Trainium2 Bass kernel for nn_MultiHeadAttention_21251498181338.

Music-Transformer-style MHA with relative position embeddings (Huang et al.
skew trick), B=2, L=2048, D=1024, H=16, causal mask.

Sharding: 8 cores = 2 batches x 4-head groups (tensor parallel per head).
Each core computes q/k/v projections for its 4 heads, causal attention with
relative position logits, and a partial output projection (Wo row-split).
Partials are summed on the host during unshard.

Device-side structure (per core):
  - Projections produce qh^T/kh^T in [head-depth on partitions] layout and
    vh in [keys on partitions] layout, so no transposes are needed anywhere
    except for the attention probabilities themselves.
  - P = exp(QK^T/8) * exp(Srel/8): the additive logit split is computed
    multiplicatively so the relative-position term can be skew-aligned
    independently of QK^T.
  - The skew is a single SBUF->SBUF DMA per (head, q-tile) using a flat
    access pattern with partition step (row_len - 1): row i is read with a
    column offset of -i, which is exactly the Huang et al. pad/reshape
    trick. Columns beyond the valid relative-index range are zeroed, which
    also implements the causal mask for free (P = Pqk * 0 = 0 there).
  - PV uses TensorE transposes of P tiles. The softmax denominators come
    for free from the fused multiply+reduce (tensor_tensor_reduce): the
    per-query row sums accumulate on the vector engine during the
    P = exp(QK)*exp(Srel) multiply, so no denominator matmuls are needed.
    The per-query reciprocals are turned into a partition-replicated
    [128, 128] tile via a DVE 32x32 stream-transpose plus two tiny
    SBUF->SBUF broadcast DMAs.
  - The two heads of each pair interleave their K=64 matmuls (different PE
    row-groups run concurrently) and share [128, P] psum tiles for PV via
    tile_position column halves, so both heads normalize in one op and
    land directly in the packed outT layout.
  - The attention output appears transposed [depth, queries], which is
    exactly the stationary-operand layout the output projection needs.
"""

import os
import sys

sys.path.insert(0, "/opt/trn_rl_repo")

import numpy as np
import ml_dtypes

import concourse.bass as bass
import concourse.mybir as mybir
import concourse.tile as tile
from concourse import bacc
from concourse.bass_utils import run_bass_kernel_spmd
from concourse.masks import make_identity

BF16 = mybir.dt.bfloat16
F32 = mybir.dt.float32
NPBF16 = ml_dtypes.bfloat16

B, L, DM, H, D = 2, 2048, 1024, 16, 64
HG = 4            # heads per core (head group)
NCORES = 8
P = 128
KT = DM // P      # 8 contraction tiles for projections
NIT = L // P      # 16 query tiles
SCALE = 1.0 / np.sqrt(D)  # 0.125

LAST_EXEC_NS = None

_PROG = None


def _ncj(it):
    # number of 512-wide key chunks for query tile `it` (causal)
    return it // 4 + 1


def build_program():
    nc = bacc.Bacc(
        "TRN2",
        target_bir_lowering=False,
        debug=False,
        enable_asserts=False,
        num_devices=NCORES,
    )

    # ---- External I/O ----
    xq = nc.dram_tensor("xq", [DM, L], BF16, kind="ExternalInput")  # q[b].T
    xk = nc.dram_tensor("xk", [DM, L], BF16, kind="ExternalInput")
    xv = nc.dram_tensor("xv", [DM, L], BF16, kind="ExternalInput")
    wq = nc.dram_tensor("wq", [DM, 2 * P], BF16, kind="ExternalInput")  # group cols
    wk = nc.dram_tensor("wk", [DM, 2 * P], BF16, kind="ExternalInput")
    wv = nc.dram_tensor("wv", [DM, 2 * P], BF16, kind="ExternalInput")
    wo = nc.dram_tensor("wo", [2, P, DM], BF16, kind="ExternalInput")  # [hp, 2h*64, dm]
    eT = nc.dram_tensor("eT", [2, P, L], BF16, kind="ExternalInput")   # [hp, 2h*64, r]
    bqk = nc.dram_tensor("bqk", [P, 6], F32, kind="ExternalInput")  # bq|bk|bv
    bo_t = nc.dram_tensor("bo", [P, DM], F32, kind="ExternalInput")     # row-replicated
    out = nc.dram_tensor("out", [L, DM], F32, kind="ExternalOutput")

    with tile.TileContext(nc) as tc:
        with (
            tc.tile_pool(name="persist", bufs=1) as pp,
            tc.tile_pool(name="small", bufs=4) as sp,
        ):
            # ---- persistent SBUF tensors ----
            ident = pp.tile([P, P], BF16)
            make_identity(nc, ident)

            wo_sb = pp.tile([P, 2, DM], BF16)
            nc.sync.dma_start(wo_sb, wo.ap().rearrange("h p m -> p h m"))
            eT_sb = pp.tile([P, 2, L], BF16)
            nc.sync.dma_start(eT_sb, eT.ap().rearrange("h p r -> p h r"))
            bqk_sb = pp.tile([P, 6], F32)
            nc.sync.dma_start(bqk_sb, bqk.ap())
            bo_sb = pp.tile([P, DM], F32)
            nc.sync.dma_start(bo_sb, bo_t.ap())

            qhT = pp.tile([P, 2, L], BF16)   # [64*hl+d, hp, i]
            khT = pp.tile([P, 2, L], BF16)
            vh = pp.tile([P, NIT, HG, 66], BF16)  # [j in tile, jt, local head, d|1|pad]
            outT = pp.tile([P, 2, L], BF16)  # [64*hl+d, hp, i]

            # exp(Srel) band buffers, one per (head, tile-in-group): each
            # use writes exp values into [0:W] and memsets the zero tail
            # [W:nj+128]; the diagonal skew DMA never reads past nj+127.
            XSE_W = 2176  # 2048 max band + 128 skew overhang
            xse_pp = [[pp.tile([P, XSE_W], BF16, name=f"xse{hl}_{i}")
                       for i in range(4)] for hl in range(2)]
            # all-ones stationary for the softmax-denominator matmul:
            # lhsT [128, 64] of ones -> psum rows all equal to the denom
            ones64 = pp.tile([P, 64], BF16)
            nc.gpsimd.memset(ones64, 1.0)

            # ---- Stage 1: projections ----
            # All three projections use the same dense N=512 streams with
            # the weight slice stationary, producing [head-depth on
            # partitions] outputs; vh's [keys on partitions] layout is then
            # recovered with 32 PE transposes.
            with (
                tc.tile_pool(name="xin", bufs=9) as xp,
                tc.tile_pool(name="s1w", bufs=1) as s1w,
                tc.tile_pool(name="ps1", bufs=4, space="PSUM") as ps1,
                tc.tile_pool(name="psvT", bufs=2, space="PSUM") as psvTp,
            ):
                wq_sb = s1w.tile([P, KT, 2 * P], BF16)
                nc.sync.dma_start(wq_sb,
                                  wq.ap().rearrange("(t p) c -> p t c", p=P))
                wk_sb = s1w.tile([P, KT, 2 * P], BF16)
                nc.sync.dma_start(wk_sb,
                                  wk.ap().rearrange("(t p) c -> p t c", p=P))
                wv_sb = s1w.tile([P, KT, 2 * P], BF16)
                nc.sync.dma_start(wv_sb,
                                  wv.ap().rearrange("(t p) c -> p t c", p=P))
                vhT = s1w.tile([P, 2, L], BF16)
                stage1 = ((xv, wv_sb, vhT, 4), (xk, wk_sb, khT, 2),
                          (xq, wq_sb, qhT, 0))
                for src, wsb, dst, bcol in stage1:
                    # per-kt chunk DMAs: matmuls start as soon as the first
                    # chunk lands instead of waiting for the full tensor
                    chunks = []
                    for kt in range(KT):
                        ch = xp.tile([P, L], BF16, tag="xchunk")
                        nc.sync.dma_start(ch, src.ap()[kt * P:(kt + 1) * P, :])
                        chunks.append(ch)
                    # kt-outer per hp: one LDWEIGHTS feeds 4 back-to-back
                    # matmuls; 4 psum tiles accumulate in parallel
                    for hp in range(2):
                        pss = {}
                        for ic in range(L // 512):
                            pss[ic] = ps1.tile([P, 512], F32, tag="ps1",
                                               name=f"ps1_{ic}")
                        for kt in range(KT):
                            for ic in range(L // 512):
                                nc.tensor.matmul(
                                    pss[ic],
                                    wsb[:, kt, hp * P:(hp + 1) * P],
                                    chunks[kt][:, ic * 512:(ic + 1) * 512],
                                    start=(kt == 0),
                                    stop=(kt == KT - 1),
                                )
                        for ic in range(L // 512):
                            nc.vector.tensor_scalar_add(
                                dst[:, hp, ic * 512:(ic + 1) * 512],
                                pss[ic],
                                bqk_sb[:, bcol + hp:bcol + hp + 1],
                            )
                    if src is xk:
                        # vhT -> vh transposes, emitted here so the k-stream
                        # masks the wait on vhT's bias adds
                        for hp in range(2):
                            for jt in range(NIT):
                                pvt = psvTp.tile([P, P], BF16, tag="psvT")
                                nc.tensor.transpose(
                                    pvt, vhT[:, hp, jt * P:(jt + 1) * P],
                                    ident)
                                nc.vector.tensor_copy(
                                    vh[:, jt, 2 * hp:2 * hp + 2, 0:64],
                                    pvt.rearrange("p (l d) -> p l d", l=2),
                                )

            # ---- Stage 2: attention, 4-q-tile supertiles ----
            # The four q-tiles of group g share the same chunk count (g+1),
            # so P^T is assembled per key-tile as a [128, 512] moving
            # operand: PV and the ones-denominator run as N=512 streams (4x
            # fewer matmuls). Masked/overhang blocks are exact zeros via the
            # skewed-band zero tail. The two heads of a pair interleave
            # their K=64 matmuls (different PE row-groups run concurrently).
            with (
                tc.tile_pool(name="pmp", bufs=2) as pmp,
                tc.tile_pool(name="psA", bufs=3, space="PSUM") as psAp,
                tc.tile_pool(name="psT", bufs=2, space="PSUM") as psTp,
                tc.tile_pool(name="psO", bufs=2, space="PSUM") as psOp,
                tc.tile_pool(name="psD", bufs=1, space="PSUM") as psDp,
            ):
                NG = NIT // 4
                # big and small groups interleaved: every scheduling window
                # contains one dense matmul stream, so the PE stays busy
                # (and HAM stays unthrottled) while the small groups' long
                # scalar/DMA chains drain.
                g_order = [NG - 1, 0, NG - 2, 1]
                for g in g_order:
                    nj = 512 * (g + 1)         # group rectangle width
                    nkt = 4 * (g + 1)          # key tiles in the rectangle
                    for hp in range(2):
                        pm = [[None] * 4 for _ in range(2)]
                        for i in range(4):
                            it = 4 * g + i
                            i0 = it * P
                            W = (it + 1) * P   # valid band width
                            r_lo = L - P - i0  # first rel index in band
                            q_stat = [qhT[64 * hl:64 * hl + 64, hp,
                                          i0:i0 + P] for hl in (0, 1)]
                            xse = [xse_pp[hl][i] for hl in (0, 1)]
                            for hl in (0, 1):
                                # zero tail [W:nj+128]: masks the causal
                                # region and pads the rectangle
                                nc.gpsimd.memset(xse[hl][:, W:nj + P], 0.0)
                            # exp(Srel/8) band [query on part, r on free]
                            for cs in range(g + 1):
                                n = min(512, W - cs * 512)
                                for hl in (0, 1):
                                    pb = 64 * hl
                                    ps = psAp.tile([P, 512], F32, tag="psA")
                                    nc.tensor.matmul(
                                        ps[:, :n],
                                        q_stat[hl],
                                        eT_sb[pb:pb + 64, hp,
                                              r_lo + cs * 512:
                                              r_lo + cs * 512 + n],
                                        start=True, stop=True,
                                    )
                                    nc.scalar.activation(
                                        xse[hl][:, cs * 512:cs * 512 + n],
                                        ps[:, :n],
                                        mybir.ActivationFunctionType.Exp,
                                        scale=SCALE,
                                    )
                            # exp(QK^T/8) chunks, skew chunks, and the
                            # P = pqk * xsk multiply, all 512 wide
                            for hl in (0, 1):
                                pm[hl][i] = pmp.tile([P, 2048], BF16,
                                                     tag=f"pm{hl}_{i}",
                                                     name=f"pm{hl}_{i}")
                            for jc in range(g + 1):
                                for hl in (0, 1):
                                    pb = 64 * hl
                                    ps = psAp.tile([P, 512], F32, tag="psA")
                                    nc.tensor.matmul(
                                        ps,
                                        q_stat[hl],
                                        khT[pb:pb + 64, hp,
                                            jc * 512:(jc + 1) * 512],
                                        start=True, stop=True,
                                    )
                                    pqc = sp.tile([P, 512], BF16,
                                                  tag=f"pqc{hl}")
                                    nc.scalar.activation(
                                        pqc, ps,
                                        mybir.ActivationFunctionType.Exp,
                                        scale=SCALE,
                                    )
                                    row_len = xse[hl].ap[0][0]
                                    diag = bass.AP(
                                        xse[hl].tensor,
                                        xse[hl].offset + 127 + jc * 512,
                                        [[row_len - 1, P], [1, 512]],
                                    )
                                    xkc = sp.tile([P, 512], BF16,
                                                  tag=f"xkc{hl}")
                                    nc.sync.dma_start(xkc, diag)
                                    nc.vector.tensor_tensor(
                                        pm[hl][i][:, jc * 512:(jc + 1) * 512],
                                        pqc, xkc, mybir.AluOpType.mult,
                                    )

                        # P^T per key tile as [128 k, 512 q] -> N=512 PV and
                        # denominator streams. Head hl owns psum partitions
                        # [64*hl, 64*hl+64); pso/psd each own a full bank
                        # (a bank tolerates only ONE open accumulation group
                        # per partition range).
                        pso = psOp.tile([P, 512], F32, tag="psO")
                        psd = psDp.tile([P, 512], F32, tag="psD")
                        for jt in range(nkt):
                            for hl in (0, 1):
                                pb = 64 * hl
                                lh = 2 * hp + hl
                                pst = psTp.tile([P, 512], BF16, tag="psT")
                                for i in range(4):
                                    nc.tensor.transpose(
                                        pst[:, i * P:(i + 1) * P],
                                        pm[hl][i][:, jt * P:(jt + 1) * P],
                                        ident,
                                    )
                                pts = sp.tile([P, 512], BF16, tag="pts")
                                nc.vector.tensor_copy(pts, pst)
                                nc.tensor.matmul(
                                    pso[pb:pb + 64, :],
                                    vh[:, jt, lh, 0:64],
                                    pts,
                                    start=(jt == 0),
                                    stop=(jt == nkt - 1),
                                    skip_group_check=True,
                                )
                                nc.tensor.matmul(
                                    psd[pb:pb + 64, :],
                                    ones64,
                                    pts,
                                    start=(jt == 0),
                                    stop=(jt == nkt - 1),
                                    skip_group_check=True,
                                )

                        # normalize both heads at once; writes land directly
                        # in the packed outT partition halves
                        rrec = sp.tile([P, 512], F32, tag="rrec")
                        nc.vector.reciprocal_approx_fast(out=rrec, in_=psd)
                        nc.vector.tensor_tensor(
                            outT[:, hp, 4 * g * P:4 * g * P + 512],
                            pso, rrec, mybir.AluOpType.mult,
                        )

            # ---- Stage 3: output projection (partial: this head group) ----
            with tc.tile_pool(name="ps3", bufs=2, space="PSUM") as ps3:
                for it in range(NIT):
                    pss3 = [ps3.tile([P, 512], F32, tag=f"ps3{mc}",
                                     name=f"ps3{mc}")
                            for mc in range(DM // 512)]
                    for hp in range(2):
                        for mc in range(DM // 512):
                            nc.tensor.matmul(
                                pss3[mc],
                                outT[:, hp, it * P:(it + 1) * P],
                                wo_sb[:, hp, mc * 512:(mc + 1) * 512],
                                start=(hp == 0),
                                stop=(hp == 1),
                            )
                    for mc in range(DM // 512):
                        osb = sp.tile([P, 512], F32, tag="osb")
                        nc.vector.tensor_tensor(
                            osb, pss3[mc], bo_sb[:, mc * 512:(mc + 1) * 512],
                            mybir.AluOpType.add,
                        )
                        nc.sync.dma_start(
                            out.ap()[it * P:(it + 1) * P, mc * 512:(mc + 1) * 512], osb
                        )
    nc.compile()
    return nc


def _prep_inputs(q, k, v, Wq, bq, Wk, bk, Wv, bv, Wo, bo, E):
    """Build the 8 per-core input maps (host-side shard + cast)."""
    in_maps = []
    for core in range(NCORES):
        b, g = core // HG, core % HG
        cols = slice(g * HG * D, (g + 1) * HG * D)  # this group's 256 cols
        # eT/wo packing: [hp, 64*hl + d, .]
        eTg = np.empty((2, P, L), NPBF16)
        wog = np.empty((2, P, DM), NPBF16)
        for hp in range(2):
            for hl in range(2):
                h = g * HG + 2 * hp + hl
                eTg[hp, 64 * hl:64 * hl + 64, :] = E[:, h * D:(h + 1) * D].T.astype(NPBF16)
                wog[hp, 64 * hl:64 * hl + 64, :] = Wo[h * D:(h + 1) * D, :].astype(NPBF16)
        bqk_a = np.empty((P, 6), np.float32)
        for hp in range(2):
            sl = slice(g * HG * D + hp * P, g * HG * D + (hp + 1) * P)
            bqk_a[:, hp] = bq[sl]
            bqk_a[:, 2 + hp] = bk[sl]
            bqk_a[:, 4 + hp] = bv[sl]
        bo_full = bo if g == 0 else np.zeros_like(bo)
        in_maps.append({
            "xq": np.ascontiguousarray(q[b].T).astype(NPBF16),
            "xk": np.ascontiguousarray(k[b].T).astype(NPBF16),
            "xv": np.ascontiguousarray(v[b].T).astype(NPBF16),
            "wq": np.ascontiguousarray(Wq[:, cols]).astype(NPBF16),
            "wk": np.ascontiguousarray(Wk[:, cols]).astype(NPBF16),
            "wv": np.ascontiguousarray(Wv[:, cols]).astype(NPBF16),
            "wo": wog,
            "eT": eTg,
            "bqk": bqk_a,
            "bo": np.ascontiguousarray(
                np.broadcast_to(bo_full[None, :], (P, DM))).astype(np.float32),
        })
    return in_maps


def _reference_numpy(q, k, v, mask, Wq, bq, Wk, bk, Wv, bv, Wo, bo, E):
    """Exact fallback for non-causal masks (never hit in practice)."""
    def split_heads(x):
        return np.moveaxis(x.reshape(*x.shape[:-1], H, D), -2, -3)
    qh = split_heads(q @ Wq + bq)
    kh = split_heads(k @ Wk + bk)
    vv = split_heads(v @ Wv + bv)
    eh = split_heads(E)
    QKt = np.einsum("bhqd,bhkd->bhqk", qh, kh)
    X = np.einsum("bhqd,hkd->bhqk", qh, eh)
    pad = np.pad(X, [(0, 0)] * 3 + [(1, 0)])
    s = pad.reshape(B, H, -1)[:, :, L:].reshape(B, H, L, L)
    logits = (QKt + s) / np.sqrt(D) + mask * -1e9
    m = logits.max(-1, keepdims=True)
    p = np.exp(logits - m)
    p /= p.sum(-1, keepdims=True)
    o = np.einsum("bhqk,bhkd->bhqd", p, vv)
    o = np.moveaxis(o, -3, -2).reshape(B, L, DM)
    return (o @ Wo + bo).astype(np.float32)


def benchmark(inputs, iters=20):
    """Amortized wall-clock of the sharded NEFF execution (device-resident
    inputs, back-to-back async dispatch). Returns est. ns per execution."""
    global _PROG
    import time as _time
    import jax
    from jax.sharding import Mesh, PartitionSpec
    from jax.experimental.shard_map import shard_map
    import concourse.bass2jax as b2j
    import concourse.mybir as mb

    if _PROG is None:
        _PROG = build_program()
    nc = _PROG
    args = {n: np.asarray(inputs[n], np.float32)
            for n in ("q", "k", "v", "Wq", "bq", "Wk", "bk", "Wv", "bv",
                      "Wo", "bo", "E")}
    in_maps = _prep_inputs(**args)
    b2j.install_neuronx_cc_hook()

    partition_name = (nc.partition_id_tensor.name
                      if nc.partition_id_tensor else None)
    in_names, out_names, out_avals, zero_outs = [], [], [], []
    for alloc in nc.m.functions[0].allocations:
        if not isinstance(alloc, mb.MemoryLocationSet):
            continue
        name = alloc.memorylocations[0].name
        if alloc.kind == "ExternalInput":
            if name != partition_name:
                in_names.append(name)
        elif alloc.kind == "ExternalOutput":
            out_names.append(name)
            shape = tuple(alloc.tensor_shape)
            dtype = mb.dt.np(alloc.dtype)
            out_avals.append(jax.core.ShapedArray(shape, dtype))
            zero_outs.append(np.zeros(shape, dtype))
    n_params = len(in_names)
    n_outs = len(out_avals)
    all_names = in_names + out_names
    if partition_name is not None:
        all_names = all_names + [partition_name]

    def _body(*fargs):
        operands = list(fargs)
        if partition_name is not None:
            operands.append(b2j.partition_id_tensor())
        outs = b2j._bass_exec_p.bind(
            *operands, out_avals=tuple(out_avals), in_names=tuple(all_names),
            out_names=tuple(out_names), lowering_input_output_aliases=(),
            sim_require_finite=True, sim_require_nnan=True, nc=nc)
        return tuple(outs)

    devices = jax.devices()[:NCORES]
    mesh = Mesh(np.asarray(devices), ("core",))
    in_specs = (PartitionSpec("core"),) * (n_params + n_outs)
    out_specs = (PartitionSpec("core"),) * n_outs
    sharded = jax.jit(
        shard_map(_body, mesh=mesh, in_specs=in_specs, out_specs=out_specs,
                  check_rep=False),
        keep_unused=True)

    concat_in = [np.concatenate([np.asarray(in_maps[c][n])
                                 for c in range(NCORES)], axis=0)
                 for n in in_names]
    dev_in = [jax.device_put(a) for a in concat_in]
    concat_zero = [np.concatenate([z] * NCORES, axis=0) for z in zero_outs]

    dev_zero = [jax.device_put(z) for z in concat_zero]
    # warmup (compiles / caches)
    outs = sharded(*dev_in, *dev_zero)
    jax.block_until_ready(outs)

    t0 = _time.perf_counter()
    results = []
    for _ in range(iters):
        results.append(sharded(*dev_in, *dev_zero))
    jax.block_until_ready(results)
    t1 = _time.perf_counter()
    return (t1 - t0) / iters * 1e9


def kernel(**inputs):
    global _PROG, LAST_EXEC_NS
    args = {n: np.asarray(inputs[n], np.float32)
            for n in ("q", "k", "v", "Wq", "bq", "Wk", "bk", "Wv", "bv",
                      "Wo", "bo", "E")}
    mask = np.asarray(inputs["mask"], np.float32)

    causal = np.array_equal(mask, np.triu(np.ones((L, L), np.float32), k=1))
    if not causal:
        return _reference_numpy(mask=mask, **args)

    if _PROG is None:
        _PROG = build_program()
    in_maps = _prep_inputs(**args)
    trace = os.environ.get("KERNEL_TRACE", "0") == "1"
    try:
        res = run_bass_kernel_spmd(_PROG, in_maps, core_ids=list(range(NCORES)),
                                   trace=trace)
    except ModuleNotFoundError:
        # axon NTFF profiling hook unavailable in this container
        res = run_bass_kernel_spmd(_PROG, in_maps, core_ids=list(range(NCORES)),
                                   trace=False)
    LAST_EXEC_NS = res.exec_time_ns
    globals()["LAST_RESULTS"] = res

    full = np.zeros((B, L, DM), np.float32)
    for core in range(NCORES):
        full[core // HG] += res.results[core]["out"]
    return full



# revision 58
# speedup vs baseline: 1.0865x; 1.0560x over previous
"""Trainium2 Bass kernel for nn_MultiHeadAttention_21251498181338.

Music-Transformer-style MHA with relative position embeddings (Huang et al.
skew trick), B=2, L=2048, D=1024, H=16, causal mask.

Sharding: 8 cores = 2 batches x 4-head groups (tensor parallel per head).
Each core computes q/k/v projections for its 4 heads, causal attention with
relative position logits, and a partial output projection (Wo row-split).
Partials are summed on the host during unshard.

Device-side structure (per core):
  - Projections produce qh^T/kh^T in [head-depth on partitions] layout and
    vh in [keys on partitions] layout, so no transposes are needed anywhere
    except for the attention probabilities themselves.
  - P = exp(QK^T/8) * exp(Srel/8): the additive logit split is computed
    multiplicatively so the relative-position term can be skew-aligned
    independently of QK^T.
  - The skew is a single SBUF->SBUF DMA per (head, q-tile) using a flat
    access pattern with partition step (row_len - 1): row i is read with a
    column offset of -i, which is exactly the Huang et al. pad/reshape
    trick. Columns beyond the valid relative-index range are zeroed, which
    also implements the causal mask for free (P = Pqk * 0 = 0 there).
  - PV uses TensorE transposes of P tiles. The softmax denominators come
    for free from the fused multiply+reduce (tensor_tensor_reduce): the
    per-query row sums accumulate on the vector engine during the
    P = exp(QK)*exp(Srel) multiply, so no denominator matmuls are needed.
    The per-query reciprocals are turned into a partition-replicated
    [128, 128] tile via a DVE 32x32 stream-transpose plus two tiny
    SBUF->SBUF broadcast DMAs.
  - The two heads of each pair interleave their K=64 matmuls (different PE
    row-groups run concurrently) and share [128, P] psum tiles for PV via
    tile_position column halves, so both heads normalize in one op and
    land directly in the packed outT layout.
  - The attention output appears transposed [depth, queries], which is
    exactly the stationary-operand layout the output projection needs.
"""

import os
import sys

sys.path.insert(0, "/opt/trn_rl_repo")

import numpy as np
import ml_dtypes

import concourse.bass as bass
import concourse.mybir as mybir
import concourse.tile as tile
from concourse import bacc
from concourse.bass_utils import run_bass_kernel_spmd
from concourse.masks import make_identity

BF16 = mybir.dt.bfloat16
F32 = mybir.dt.float32
NPBF16 = ml_dtypes.bfloat16

B, L, DM, H, D = 2, 2048, 1024, 16, 64
HG = 4            # heads per core (head group)
NCORES = 8
P = 128
KT = DM // P      # 8 contraction tiles for projections
NIT = L // P      # 16 query tiles
SCALE = 1.0 / np.sqrt(D)  # 0.125

LAST_EXEC_NS = None

_PROG = None


def _ncj(it):
    # number of 512-wide key chunks for query tile `it` (causal)
    return it // 4 + 1


def build_program():
    nc = bacc.Bacc(
        "TRN2",
        target_bir_lowering=False,
        debug=False,
        enable_asserts=False,
        num_devices=NCORES,
    )

    # ---- External I/O ----
    xq = nc.dram_tensor("xq", [DM, L], BF16, kind="ExternalInput")  # q[b].T
    xk = nc.dram_tensor("xk", [DM, L], BF16, kind="ExternalInput")
    xv = nc.dram_tensor("xv", [DM, L], BF16, kind="ExternalInput")
    wq = nc.dram_tensor("wq", [DM, 2 * P], BF16, kind="ExternalInput")  # group cols
    wk = nc.dram_tensor("wk", [DM, 2 * P], BF16, kind="ExternalInput")
    wv = nc.dram_tensor("wv", [DM, 2 * P], BF16, kind="ExternalInput")
    wo = nc.dram_tensor("wo", [2, P, DM], BF16, kind="ExternalInput")  # [hp, 2h*64, dm]
    eT = nc.dram_tensor("eT", [2, P, L], BF16, kind="ExternalInput")   # [hp, 2h*64, r]
    bqk = nc.dram_tensor("bqk", [P, 6], F32, kind="ExternalInput")  # bq|bk|bv
    bo_t = nc.dram_tensor("bo", [P, DM], F32, kind="ExternalInput")     # row-replicated
    out = nc.dram_tensor("out", [L, DM], F32, kind="ExternalOutput")

    with tile.TileContext(nc) as tc:
        with (
            tc.tile_pool(name="persist", bufs=1) as pp,
            tc.tile_pool(name="small", bufs=4) as sp,
        ):
            # ---- persistent SBUF tensors ----
            ident = pp.tile([P, P], BF16)
            make_identity(nc, ident)

            wo_sb = pp.tile([P, 2, DM], BF16)
            nc.sync.dma_start(wo_sb, wo.ap().rearrange("h p m -> p h m"))
            eT_sb = pp.tile([P, 2, L], BF16)
            nc.sync.dma_start(eT_sb, eT.ap().rearrange("h p r -> p h r"))
            bqk_sb = pp.tile([P, 6], F32)
            nc.sync.dma_start(bqk_sb, bqk.ap())
            bo_sb = pp.tile([P, DM], F32)
            nc.sync.dma_start(bo_sb, bo_t.ap())

            qhT = pp.tile([P, 2, L], BF16)   # [64*hl+d, hp, i]
            khT = pp.tile([P, 2, L], BF16)
            vh = pp.tile([P, NIT, HG, 66], BF16)  # [j in tile, jt, local head, d|1|pad]
            outT = pp.tile([P, 2, L], BF16)  # [64*hl+d, hp, i]

            # exp(Srel) band buffers, one per (head, tile-in-group): each
            # use writes exp values into [0:W] and memsets the zero tail
            # [W:nj+128]; the diagonal skew DMA never reads past nj+127.
            XSE_W = 2176  # 2048 max band + 128 skew overhang
            xse_pp = [[pp.tile([P, XSE_W], BF16, name=f"xse{hl}_{i}")
                       for i in range(4)] for hl in range(2)]
            # all-ones stationary for the softmax-denominator matmul:
            # lhsT [128, 64] of ones -> psum rows all equal to the denom
            ones64 = pp.tile([P, 64], BF16)
            nc.gpsimd.memset(ones64, 1.0)

            # ---- Stage 1: projections ----
            # All three projections use the same dense N=512 streams with
            # the weight slice stationary, producing [head-depth on
            # partitions] outputs; vh's [keys on partitions] layout is then
            # recovered with 32 PE transposes.
            with (
                tc.tile_pool(name="xin", bufs=9) as xp,
                tc.tile_pool(name="s1w", bufs=1) as s1w,
                tc.tile_pool(name="ps1", bufs=4, space="PSUM") as ps1,
                tc.tile_pool(name="psvT", bufs=2, space="PSUM") as psvTp,
            ):
                wq_sb = s1w.tile([P, KT, 2 * P], BF16)
                nc.sync.dma_start(wq_sb,
                                  wq.ap().rearrange("(t p) c -> p t c", p=P))
                wk_sb = s1w.tile([P, KT, 2 * P], BF16)
                nc.sync.dma_start(wk_sb,
                                  wk.ap().rearrange("(t p) c -> p t c", p=P))
                wv_sb = s1w.tile([P, KT, 2 * P], BF16)
                nc.sync.dma_start(wv_sb,
                                  wv.ap().rearrange("(t p) c -> p t c", p=P))
                vhT = s1w.tile([P, 2, L], BF16)
                stage1 = ((xv, wv_sb, vhT, 4), (xk, wk_sb, khT, 2),
                          (xq, wq_sb, qhT, 0))
                for src, wsb, dst, bcol in stage1:
                    # per-kt chunk DMAs: matmuls start as soon as the first
                    # chunk lands instead of waiting for the full tensor
                    chunks = []
                    for kt in range(KT):
                        ch = xp.tile([P, L], BF16, tag="xchunk")
                        nc.sync.dma_start(ch, src.ap()[kt * P:(kt + 1) * P, :])
                        chunks.append(ch)
                    # kt-outer per hp: one LDWEIGHTS feeds 4 back-to-back
                    # matmuls; 4 psum tiles accumulate in parallel
                    for hp in range(2):
                        pss = {}
                        for ic in range(L // 512):
                            pss[ic] = ps1.tile([P, 512], F32, tag="ps1",
                                               name=f"ps1_{ic}")
                        for kt in range(KT):
                            for ic in range(L // 512):
                                nc.tensor.matmul(
                                    pss[ic],
                                    wsb[:, kt, hp * P:(hp + 1) * P],
                                    chunks[kt][:, ic * 512:(ic + 1) * 512],
                                    start=(kt == 0),
                                    stop=(kt == KT - 1),
                                )
                        for ic in range(L // 512):
                            nc.vector.tensor_scalar_add(
                                dst[:, hp, ic * 512:(ic + 1) * 512],
                                pss[ic],
                                bqk_sb[:, bcol + hp:bcol + hp + 1],
                            )
                    if src is xk:
                        # vhT -> vh transposes, emitted here so the k-stream
                        # masks the wait on vhT's bias adds
                        for hp in range(2):
                            for jt in range(NIT):
                                pvt = psvTp.tile([P, P], BF16, tag="psvT")
                                nc.tensor.transpose(
                                    pvt, vhT[:, hp, jt * P:(jt + 1) * P],
                                    ident)
                                nc.vector.tensor_copy(
                                    vh[:, jt, 2 * hp:2 * hp + 2, 0:64],
                                    pvt.rearrange("p (l d) -> p l d", l=2),
                                )

            # ---- Stage 2: attention ----
            # The two heads of a pair are interleaved so adjacent K=64
            # matmuls target different PE row-groups and run concurrently.
            with (
                tc.tile_pool(name="pmp", bufs=2) as pmp,
                tc.tile_pool(name="psA", bufs=3, space="PSUM") as psAp,
                tc.tile_pool(name="psT", bufs=2, space="PSUM") as psTp,
                tc.tile_pool(name="psO", bufs=2, space="PSUM") as psOp,
                tc.tile_pool(name="psD", bufs=1, space="PSUM") as psDp,
            ):
                # big and small iterations interleaved one-for-one: every
                # other (it, hp) step carries a dense matmul stream, so the
                # PE stays busy (and HAM unthrottled) while the small tiles'
                # long scalar/DMA chains drain.
                seq = []
                for i in range(NIT // 2):
                    for hp in range(2):
                        seq.append((NIT - 1 - i, hp))
                        seq.append((i, hp))
                for itc, (it, hp) in enumerate(seq):
                    nkt = it + 1               # valid 128-key tiles
                    W = nkt * P                # valid band width (r cols)
                    ncj = (W + 511) // 512     # 512-wide chunks (last partial)
                    i0 = it * P
                    r_lo = L - P - i0          # first relative index in band
                    q_stat = [qhT[64 * hl:64 * hl + 64, hp, i0:i0 + P]
                              for hl in (0, 1)]
                    xse = [xse_pp[hl][itc % 4] for hl in (0, 1)]
                    for hl in (0, 1):
                        # zero the 128-col skew overhang; cols beyond
                        # W+127 are never read by the diagonal DMA
                        nc.gpsimd.memset(xse[hl][:, W:W + P], 0.0)

                    # exp(Srel/8) bands, [query on partitions, r on free]
                    for cs in range(ncj):
                        n = min(512, W - cs * 512)
                        for hl in (0, 1):
                            pb = 64 * hl
                            ps = psAp.tile([P, 512], F32, tag="psA")
                            nc.tensor.matmul(
                                ps[:, :n],
                                q_stat[hl],
                                eT_sb[pb:pb + 64, hp,
                                      r_lo + cs * 512:r_lo + cs * 512 + n],
                                start=True, stop=True,
                            )
                            nc.scalar.activation(
                                xse[hl][:, cs * 512:cs * 512 + n], ps[:, :n],
                                mybir.ActivationFunctionType.Exp, scale=SCALE,
                            )

                    # exp(QK^T/8), trimmed to the valid band
                    pqk = [pmp.tile([P, 2048], BF16, tag=f"pqk{hl}",
                                    name=f"pqk{hl}") for hl in (0, 1)]
                    for jc in range(ncj):
                        n = min(512, W - jc * 512)
                        for hl in (0, 1):
                            pb = 64 * hl
                            ps = psAp.tile([P, 512], F32, tag="psA")
                            nc.tensor.matmul(
                                ps[:, :n],
                                q_stat[hl],
                                khT[pb:pb + 64, hp, jc * 512:jc * 512 + n],
                                start=True, stop=True,
                            )
                            nc.scalar.activation(
                                pqk[hl][:, jc * 512:jc * 512 + n], ps[:, :n],
                                mybir.ActivationFunctionType.Exp, scale=SCALE,
                            )

                    # skew (SBUF->SBUF diagonal DMA) and P = pqk * xsk
                    pm = []
                    for hl in (0, 1):
                        xsk = pmp.tile([P, 2048], BF16, tag=f"xsk{hl}",
                                       name=f"xsk{hl}")
                        row_len = xse[hl].ap[0][0]
                        diag = bass.AP(
                            xse[hl].tensor, xse[hl].offset + 127,
                            [[row_len - 1, P], [1, W]],
                        )
                        nc.sync.dma_start(xsk[:, :W], diag)
                        pmt = pmp.tile([P, 2048], BF16, tag=f"pm{hl}",
                                       name=f"pm{hl}")
                        nc.vector.tensor_tensor(
                            pmt[:, :W], pqk[hl][:, :W], xsk[:, :W],
                            mybir.AluOpType.mult,
                        )
                        pm.append(pmt)

                    # PV + denominator into column-half psum tiles: head hl
                    # occupies psum partitions [64*hl, 64*hl+64). pso/psd
                    # own separate banks (a bank tolerates only ONE open
                    # accumulation group per partition range).
                    pso = psOp.tile([P, P], F32, tag="psO")
                    psd = psDp.tile([P, P], F32, tag="psD")
                    for jb in range(ncj):
                        w_jb = min(512, W - jb * 512)
                        ntk = (w_jb + P - 1) // P
                        for hl in (0, 1):
                            pb = 64 * hl
                            lh = 2 * hp + hl
                            pst = psTp.tile([P, 512], BF16, tag="psT")
                            for t in range(ntk):
                                nc.tensor.transpose(
                                    pst[:, t * P:(t + 1) * P],
                                    pm[hl][:, jb * 512 + t * P:
                                           jb * 512 + (t + 1) * P],
                                    ident,
                                )
                            pts = sp.tile([P, 512], BF16, tag="pts")
                            nc.vector.tensor_copy(pts[:, :w_jb],
                                                  pst[:, :w_jb])
                            for t in range(ntk):
                                jt = jb * 4 + t
                                nc.tensor.matmul(
                                    pso[pb:pb + 64, :],
                                    vh[:, jt, lh, 0:64],
                                    pts[:, t * P:(t + 1) * P],
                                    start=(jt == 0),
                                    stop=(jt == nkt - 1),
                                    skip_group_check=True,
                                )
                                nc.tensor.matmul(
                                    psd[pb:pb + 64, :],
                                    ones64,
                                    pts[:, t * P:(t + 1) * P],
                                    start=(jt == 0),
                                    stop=(jt == nkt - 1),
                                    skip_group_check=True,
                                )

                    # normalize both heads at once; writes land directly
                    # in the packed outT partition halves
                    rrec = sp.tile([P, P], F32, tag="rrec")
                    nc.vector.reciprocal_approx_fast(out=rrec, in_=psd)
                    nc.vector.tensor_tensor(
                        outT[:, hp, i0:i0 + P], pso, rrec,
                        mybir.AluOpType.mult,
                    )

            # ---- Stage 3: output projection (partial: this head group) ----
            with tc.tile_pool(name="ps3", bufs=2, space="PSUM") as ps3:
                for it in range(NIT):
                    pss3 = [ps3.tile([P, 512], F32, tag=f"ps3{mc}",
                                     name=f"ps3{mc}")
                            for mc in range(DM // 512)]
                    for hp in range(2):
                        for mc in range(DM // 512):
                            nc.tensor.matmul(
                                pss3[mc],
                                outT[:, hp, it * P:(it + 1) * P],
                                wo_sb[:, hp, mc * 512:(mc + 1) * 512],
                                start=(hp == 0),
                                stop=(hp == 1),
                            )
                    for mc in range(DM // 512):
                        osb = sp.tile([P, 512], F32, tag="osb")
                        nc.vector.tensor_tensor(
                            osb, pss3[mc], bo_sb[:, mc * 512:(mc + 1) * 512],
                            mybir.AluOpType.add,
                        )
                        nc.sync.dma_start(
                            out.ap()[it * P:(it + 1) * P, mc * 512:(mc + 1) * 512], osb
                        )
    nc.compile()
    return nc


def _prep_inputs(q, k, v, Wq, bq, Wk, bk, Wv, bv, Wo, bo, E):
    """Build the 8 per-core input maps (host-side shard + cast)."""
    in_maps = []
    for core in range(NCORES):
        b, g = core // HG, core % HG
        cols = slice(g * HG * D, (g + 1) * HG * D)  # this group's 256 cols
        # eT/wo packing: [hp, 64*hl + d, .]
        eTg = np.empty((2, P, L), NPBF16)
        wog = np.empty((2, P, DM), NPBF16)
        for hp in range(2):
            for hl in range(2):
                h = g * HG + 2 * hp + hl
                eTg[hp, 64 * hl:64 * hl + 64, :] = E[:, h * D:(h + 1) * D].T.astype(NPBF16)
                wog[hp, 64 * hl:64 * hl + 64, :] = Wo[h * D:(h + 1) * D, :].astype(NPBF16)
        bqk_a = np.empty((P, 6), np.float32)
        for hp in range(2):
            sl = slice(g * HG * D + hp * P, g * HG * D + (hp + 1) * P)
            bqk_a[:, hp] = bq[sl]
            bqk_a[:, 2 + hp] = bk[sl]
            bqk_a[:, 4 + hp] = bv[sl]
        bo_full = bo if g == 0 else np.zeros_like(bo)
        in_maps.append({
            "xq": np.ascontiguousarray(q[b].T).astype(NPBF16),
            "xk": np.ascontiguousarray(k[b].T).astype(NPBF16),
            "xv": np.ascontiguousarray(v[b].T).astype(NPBF16),
            "wq": np.ascontiguousarray(Wq[:, cols]).astype(NPBF16),
            "wk": np.ascontiguousarray(Wk[:, cols]).astype(NPBF16),
            "wv": np.ascontiguousarray(Wv[:, cols]).astype(NPBF16),
            "wo": wog,
            "eT": eTg,
            "bqk": bqk_a,
            "bo": np.ascontiguousarray(
                np.broadcast_to(bo_full[None, :], (P, DM))).astype(np.float32),
        })
    return in_maps


def _reference_numpy(q, k, v, mask, Wq, bq, Wk, bk, Wv, bv, Wo, bo, E):
    """Exact fallback for non-causal masks (never hit in practice)."""
    def split_heads(x):
        return np.moveaxis(x.reshape(*x.shape[:-1], H, D), -2, -3)
    qh = split_heads(q @ Wq + bq)
    kh = split_heads(k @ Wk + bk)
    vv = split_heads(v @ Wv + bv)
    eh = split_heads(E)
    QKt = np.einsum("bhqd,bhkd->bhqk", qh, kh)
    X = np.einsum("bhqd,hkd->bhqk", qh, eh)
    pad = np.pad(X, [(0, 0)] * 3 + [(1, 0)])
    s = pad.reshape(B, H, -1)[:, :, L:].reshape(B, H, L, L)
    logits = (QKt + s) / np.sqrt(D) + mask * -1e9
    m = logits.max(-1, keepdims=True)
    p = np.exp(logits - m)
    p /= p.sum(-1, keepdims=True)
    o = np.einsum("bhqk,bhkd->bhqd", p, vv)
    o = np.moveaxis(o, -3, -2).reshape(B, L, DM)
    return (o @ Wo + bo).astype(np.float32)


def benchmark(inputs, iters=20):
    """Amortized wall-clock of the sharded NEFF execution (device-resident
    inputs, back-to-back async dispatch). Returns est. ns per execution."""
    global _PROG
    import time as _time
    import jax
    from jax.sharding import Mesh, PartitionSpec
    from jax.experimental.shard_map import shard_map
    import concourse.bass2jax as b2j
    import concourse.mybir as mb

    if _PROG is None:
        _PROG = build_program()
    nc = _PROG
    args = {n: np.asarray(inputs[n], np.float32)
            for n in ("q", "k", "v", "Wq", "bq", "Wk", "bk", "Wv", "bv",
                      "Wo", "bo", "E")}
    in_maps = _prep_inputs(**args)
    b2j.install_neuronx_cc_hook()

    partition_name = (nc.partition_id_tensor.name
                      if nc.partition_id_tensor else None)
    in_names, out_names, out_avals, zero_outs = [], [], [], []
    for alloc in nc.m.functions[0].allocations:
        if not isinstance(alloc, mb.MemoryLocationSet):
            continue
        name = alloc.memorylocations[0].name
        if alloc.kind == "ExternalInput":
            if name != partition_name:
                in_names.append(name)
        elif alloc.kind == "ExternalOutput":
            out_names.append(name)
            shape = tuple(alloc.tensor_shape)
            dtype = mb.dt.np(alloc.dtype)
            out_avals.append(jax.core.ShapedArray(shape, dtype))
            zero_outs.append(np.zeros(shape, dtype))
    n_params = len(in_names)
    n_outs = len(out_avals)
    all_names = in_names + out_names
    if partition_name is not None:
        all_names = all_names + [partition_name]

    def _body(*fargs):
        operands = list(fargs)
        if partition_name is not None:
            operands.append(b2j.partition_id_tensor())
        outs = b2j._bass_exec_p.bind(
            *operands, out_avals=tuple(out_avals), in_names=tuple(all_names),
            out_names=tuple(out_names), lowering_input_output_aliases=(),
            sim_require_finite=True, sim_require_nnan=True, nc=nc)
        return tuple(outs)

    devices = jax.devices()[:NCORES]
    mesh = Mesh(np.asarray(devices), ("core",))
    in_specs = (PartitionSpec("core"),) * (n_params + n_outs)
    out_specs = (PartitionSpec("core"),) * n_outs
    sharded = jax.jit(
        shard_map(_body, mesh=mesh, in_specs=in_specs, out_specs=out_specs,
                  check_rep=False),
        keep_unused=True)

    concat_in = [np.concatenate([np.asarray(in_maps[c][n])
                                 for c in range(NCORES)], axis=0)
                 for n in in_names]
    dev_in = [jax.device_put(a) for a in concat_in]
    concat_zero = [np.concatenate([z] * NCORES, axis=0) for z in zero_outs]

    dev_zero = [jax.device_put(z) for z in concat_zero]
    # warmup (compiles / caches)
    outs = sharded(*dev_in, *dev_zero)
    jax.block_until_ready(outs)

    t0 = _time.perf_counter()
    results = []
    for _ in range(iters):
        results.append(sharded(*dev_in, *dev_zero))
    jax.block_until_ready(results)
    t1 = _time.perf_counter()
    return (t1 - t0) / iters * 1e9


def kernel(**inputs):
    global _PROG, LAST_EXEC_NS
    args = {n: np.asarray(inputs[n], np.float32)
            for n in ("q", "k", "v", "Wq", "bq", "Wk", "bk", "Wv", "bv",
                      "Wo", "bo", "E")}
    mask = np.asarray(inputs["mask"], np.float32)

    causal = np.array_equal(mask, np.triu(np.ones((L, L), np.float32), k=1))
    if not causal:
        return _reference_numpy(mask=mask, **args)

    if _PROG is None:
        _PROG = build_program()
    in_maps = _prep_inputs(**args)
    trace = os.environ.get("KERNEL_TRACE", "0") == "1"
    try:
        res = run_bass_kernel_spmd(_PROG, in_maps, core_ids=list(range(NCORES)),
                                   trace=trace)
    except ModuleNotFoundError:
        # axon NTFF profiling hook unavailable in this container
        res = run_bass_kernel_spmd(_PROG, in_maps, core_ids=list(range(NCORES)),
                                   trace=False)
    LAST_EXEC_NS = res.exec_time_ns
    globals()["LAST_RESULTS"] = res

    full = np.zeros((B, L, DM), np.float32)
    for core in range(NCORES):
        full[core // HG] += res.results[core]["out"]
    return full

